# revision 1
# baseline (speedup 1.0000x reference)
"""Trainium2 Bass kernel for nn_DCTLayer: per-8x8-block 2D DCT-like transform.

Math: reference computes, per 8x8 block X of the 256x256 image,
    out_block[y, v] = sum_x A[v, x] * X[x, y],   where A = D @ D
(D = 8x8 DCT basis). out_block = (A @ X)^T.

Kernel strategy (per core, pure data parallel over batch):
  - Load 128 consecutive image rows into SBUF naturally: partition = (G, x)
    [G = row-block, x = row-within-block], free = (J, y) [J = col-block,
    y = col-within-block].  Fully contiguous DMA.
  - One matmul per 128x128 tile with the DATA as the stationary operand and a
    constant 128x128 block-diagonal matrix BD (16 copies of A^T on the
    diagonal) as the moving operand:
        Z[(J,y), (G,v)] = sum_{G,x} X[(G,x),(J,y)] * A[v,x]
  - Store Z to DRAM with a 4D strided access pattern that places element
    (J,y,G,v) at output row 8G+y, col 8J+v.  This performs the within-block
    transpose during the store (32-byte contiguous runs).
"""

import sys

sys.path.insert(0, "/opt/trn_rl_repo")

from contextlib import ExitStack

import numpy as np

import concourse.bass as bass  # noqa: F401
import concourse.tile as tile
from concourse import bacc, mybir
from concourse.bass_utils import run_bass_kernel_spmd

P = 8
H = W = 256
B, C = 16, 64
NCORES = 8
BPC = B // NCORES  # batches per core
IMGS = BPC * C  # images (b,c planes) per core
ROWS = IMGS * H  # dram rows per core

TRACE = False
LAST_RESULTS = None

_nc_cache = None


def _ensure_ntff_hook():
    """The agent image's antenv lacks axon_hooks; synthesize it so
    run_bass_kernel_spmd(trace=True) can capture NTFF profiles."""
    import types

    if "antenv.axon_hooks" in sys.modules:
        return
    try:
        sys.path.insert(0, "/root/.axon_site/trn_agent_boot")
        from trn_boot import _ntff_profile_via_ctypes

        hook = _ntff_profile_via_ctypes("/opt/axon/libaxon_pjrt.so")
    except Exception:
        hook = None
    mod = types.ModuleType("antenv.axon_hooks")
    mod._hook = hook
    mod.get_axon_ntff_profile_hook = lambda: mod._hook
    mod.set_axon_ntff_profile_hook = lambda h: setattr(mod, "_hook", h)
    sys.modules["antenv.axon_hooks"] = mod


def _dct_kernel(tc, o, x, bd):
    nc = tc.nc
    with ExitStack() as ctx:
        xpool = ctx.enter_context(tc.tile_pool(name="xin", bufs=10))
        zpool = ctx.enter_context(tc.tile_pool(name="zout", bufs=10))
        cpool = ctx.enter_context(tc.tile_pool(name="const", bufs=1))
        ppool = ctx.enter_context(tc.tile_pool(name="ps", bufs=8, space="PSUM"))

        bdt = cpool.tile([128, 128], mybir.dt.float32)
        nc.sync.dma_start(bdt[:], bd[:])

        for img in range(IMGS):
            # ---- load image (256x256) as [128, (r, c)] ----
            xt = xpool.tile([128, 2 * W], mybir.dt.float32)
            src = x[img * H : (img + 1) * H, :].rearrange("(r p) c -> p r c", p=128)
            dst = xt[:].rearrange("p (r c) -> p r c", c=W)
            nc.gpsimd.dma_start(dst, src)

            # ---- 4 matmuls into one PSUM bank: quarter q = (r, h) ----
            ps = ppool.tile([128, 512], mybir.dt.float32)
            for r in range(2):
                for h in range(2):
                    q = r * 2 + h
                    nc.tensor.matmul(
                        ps[:, q * 128 : (q + 1) * 128],
                        xt[:, r * W + h * 128 : r * W + (h + 1) * 128],
                        bdt[:],
                        start=True,
                        stop=True,
                    )

            # ---- PSUM -> SBUF with (r,h) -> (h,r) quarter swap ----
            zt = zpool.tile([128, 512], mybir.dt.float32)
            zsrc = ps[:].rearrange("p (r h c) -> p r h c", r=2, h=2)
            zdst = zt[:].rearrange("p (h r c) -> p r h c", h=2, r=2)
            nc.vector.tensor_copy(zdst, zsrc)

            # ---- strided store performing the within-block transpose ----
            # zt half h: [p=(J,y), f=(G,v)] with G = 0..31 over both row-chunks
            # DMA APs are limited to 3 dims -> one DMA per y (rows 8G+y).
            # split descriptor generation across all three DGE issuers.
            # HWDGE (SP/ACT) ~1.25ns/desc, SWDGE (Pool) ~3ns/desc -> 7/7/2.
            for h in range(2):
                for y in range(P):
                    i = h * P + y
                    if i < 6:
                        eng = nc.sync
                    elif i < 12:
                        eng = nc.scalar
                    else:
                        eng = nc.gpsimd
                    dstore = o[
                        img * H + y : (img + 1) * H : P, h * 128 : (h + 1) * 128
                    ].rearrange("G (J v) -> J G v", v=P)
                    eng.dma_start(dstore, zt[y::P, h * 256 : (h + 1) * 256])


def _build_nc():
    nc = bacc.Bacc(
        "TRN2", target_bir_lowering=False, debug=False, num_devices=NCORES
    )
    x_ap = nc.dram_tensor("x", [ROWS, W], mybir.dt.float32, kind="ExternalInput").ap()
    bd_ap = nc.dram_tensor(
        "bd", [128, 128], mybir.dt.float32, kind="ExternalInput"
    ).ap()
    o_ap = nc.dram_tensor("o", [ROWS, W], mybir.dt.float32, kind="ExternalOutput").ap()
    with tile.TileContext(nc) as tc:
        _dct_kernel(tc, o_ap, x_ap, bd_ap)
    nc.compile()
    return nc


def _make_bd(dct_basis: np.ndarray) -> np.ndarray:
    a = dct_basis.astype(np.float64) @ dct_basis.astype(np.float64)
    at = a.T.astype(np.float32)  # block[x, v] = A[v, x]
    bd = np.zeros((128, 128), dtype=np.float32)
    for g in range(16):
        bd[g * P : (g + 1) * P, g * P : (g + 1) * P] = at
    return bd


def kernel(x: np.ndarray, dct_basis: np.ndarray) -> np.ndarray:
    global _nc_cache, LAST_RESULTS
    x = np.asarray(x, dtype=np.float32)
    dct_basis = np.asarray(dct_basis, dtype=np.float32)
    assert x.shape == (B, C, H, W)

    if _nc_cache is None:
        _nc_cache = _build_nc()
    nc = _nc_cache

    bd = _make_bd(dct_basis)
    in_maps = []
    for i in range(NCORES):
        xs = np.ascontiguousarray(x[i * BPC : (i + 1) * BPC]).reshape(ROWS, W)
        in_maps.append({"x": xs, "bd": bd})

    if TRACE:
        _ensure_ntff_hook()
    try:
        res = run_bass_kernel_spmd(
            nc, in_maps, core_ids=list(range(NCORES)), trace=TRACE
        )
    except ModuleNotFoundError:
        res = run_bass_kernel_spmd(
            nc, in_maps, core_ids=list(range(NCORES)), trace=False
        )
    LAST_RESULTS = res

    out = np.empty((B, C, H, W), dtype=np.float32)
    for i in range(NCORES):
        out[i * BPC : (i + 1) * BPC] = res.results[i]["o"].reshape(BPC, C, H, W)
    return out



# revision 10
# speedup vs baseline: 2.1377x; 2.1377x over previous
"""Trainium2 Bass kernel for nn_DCTLayer: per-8x8-block 2D DCT-like transform.

Math: reference computes, per 8x8 block X of the 256x256 image,
    out_block[y, v] = sum_x A[v, x] * X[x, y],   where A = D @ D
(D = 8x8 DCT basis). out_block = (A @ X)^T.

Kernel strategy (per core, pure data parallel over batch):
  - Load 128 consecutive image rows into SBUF naturally: partition = (G, x)
    [p = 8G+x], free = (r, c) [r = row-half, c = column]. Contiguous 1KB-run
    DMA.
  - Matmul per (r, h): stationary = data with free AP ordered (y, j)
    [c = 128h + 8j + y], moving = constant BD2[8g+x, 16v+g] = A[v,x].
    PSUM out: partition (y, j) [p = 16y+j], written STRIDED so the h-half of
    PSUM has free layout f = 32v + 16r + g.
  - DVE stream-transpose (32x32 blocks) per h-half: swaps partition-lo5
    (y0, j) with free-lo5 (r, g), yielding partition p' = 32Y + 16r + g
    [Y = y>>1], free (v, y0, j).  This is the cross-partition shuffle that
    makes output rows contiguous per partition.
  - Reorder copy (v, y0, j) -> st2 layout (y0, h, j, v): now partition p'
    holds output rows 8q + 2Y + y0 (q = p' mod 32) as full 1KB spans.
  - Store per Y-group: 3-dim AP [q(32 partitions), y0, 1KB row run];
    row = 8q + 2Y + y0 is affine in the partition index.  4 store DMAs per
    image with 1KB contiguous runs (full DMA bandwidth) vs the old 16 DMAs
    of 32B runs.
"""

import sys

sys.path.insert(0, "/opt/trn_rl_repo")

from contextlib import ExitStack

import numpy as np

import concourse.bass as bass  # noqa: F401
import concourse.tile as tile
from concourse import bacc, mybir
from concourse.bass_utils import run_bass_kernel_spmd

P = 8
H = W = 256
B, C = 16, 64
NCORES = 8
BPC = B // NCORES  # batches per core
IMGS = BPC * C  # images (b,c planes) per core
ROWS = IMGS * H  # dram rows per core

TRACE = False
LAST_RESULTS = None

_nc_cache = None


def _ensure_ntff_hook():
    """The agent image's antenv lacks axon_hooks; synthesize it so
    run_bass_kernel_spmd(trace=True) can capture NTFF profiles."""
    import types

    if "antenv.axon_hooks" in sys.modules:
        return
    try:
        sys.path.insert(0, "/root/.axon_site/trn_agent_boot")
        from trn_boot import _ntff_profile_via_ctypes

        hook = _ntff_profile_via_ctypes("/opt/axon/libaxon_pjrt.so")
    except Exception:
        hook = None
    mod = types.ModuleType("antenv.axon_hooks")
    mod._hook = hook
    mod.get_axon_ntff_profile_hook = lambda: mod._hook
    mod.set_axon_ntff_profile_hook = lambda h: setattr(mod, "_hook", h)
    sys.modules["antenv.axon_hooks"] = mod


def _dct_kernel(tc, o, x, bd):
    nc = tc.nc
    with ExitStack() as ctx:
        xpool = ctx.enter_context(tc.tile_pool(name="xin", bufs=6))
        x2pool = ctx.enter_context(tc.tile_pool(name="xperm", bufs=4))
        zpool = ctx.enter_context(tc.tile_pool(name="zint", bufs=4))
        spool = ctx.enter_context(tc.tile_pool(name="stt", bufs=4))
        s2pool = ctx.enter_context(tc.tile_pool(name="st2", bufs=4))
        cpool = ctx.enter_context(tc.tile_pool(name="const", bufs=1))
        ppool = ctx.enter_context(tc.tile_pool(name="ps", bufs=4, space="PSUM"))

        bdt = cpool.tile([128, 128], mybir.dt.float32)
        nc.sync.dma_start(bdt[:], bd[:])

        for img in range(IMGS):
            # ---- load image (256x256) as [p=(G,x), (r, c)]; 1KB runs ----
            xt = xpool.tile([128, 2 * W], mybir.dt.float32)
            src = x[img * H : (img + 1) * H, :].rearrange("(r p) c -> p r c", p=128)
            dst = xt[:].rearrange("p (r c) -> p r c", c=W)
            nc.sync.dma_start(dst, src)

            # ---- pre-permute cols to y-major so matmul APs stay 1-D ----
            # xt  free: (r, h, j, y)   ->   xt2 free: (r, h, y, j)
            xt2 = x2pool.tile([128, 2 * W], mybir.dt.float32)
            for r in range(2):
                cin = xt[:, r * W : (r + 1) * W].rearrange(
                    "p (h j y) -> p h y j", h=2, j=16, y=8
                )
                cout = xt2[:, r * W : (r + 1) * W].rearrange(
                    "p (h y j) -> p h y j", h=2, y=8, j=16
                )
                nc.vector.tensor_copy(cout, cin)

            # ---- 4 plain matmuls: psum chunk (r,h) = [p=(y,j), f=(v,g)] ----
            ps = ppool.tile([128, 512], mybir.dt.float32)
            for r in range(2):
                for h in range(2):
                    stat = xt2[:, r * W + h * 128 : r * W + (h + 1) * 128]
                    pout = ps[:, r * 256 + h * 128 : r * 256 + h * 128 + 128]
                    nc.tensor.matmul(pout, stat, bdt[:], start=True, stop=True)

            # ---- interleave copy: psum chunk (r,h) (v,g) -> zt 32v+16r+g ----
            # v-stride 32 needs a 224-col (7x32) window; v=7 tail separately.
            zt = zpool.tile([128, 512], mybir.dt.float32)
            for h in range(2):
                ceng = nc.scalar if h == 0 else nc.vector
                for r in range(2):
                    chunk = ps[:, r * 256 + h * 128 : r * 256 + h * 128 + 128]
                    base2 = h * 256 + r * 16
                    cin = chunk[:, 0:112].rearrange("p (v g) -> p v g", v=7, g=16)
                    cout = zt[:, base2 : base2 + 224].rearrange(
                        "p (v w) -> p v w", v=7, w=32
                    )[:, :, 0:16]
                    tail_in = chunk[:, 112:128]
                    tail_out = zt[:, base2 + 224 : base2 + 240]
                    if ceng is nc.scalar:
                        ceng.mul(cout, cin, 1.0)
                        ceng.mul(tail_out, tail_in, 1.0)
                    else:
                        ceng.tensor_copy(cout, cin)
                        ceng.tensor_copy(tail_out, tail_in)

            # ---- DVE stream transpose per h-half: [128,256] plain 2D ----
            # in:  partition (Y, y0, j), free (v, r, g)
            # out: partition (Y, r, g),  free (v, y0, j)
            st = spool.tile([128, 512], mybir.dt.float32)
            for h in range(2):
                nc.vector.transpose(
                    st[:, h * 256 : (h + 1) * 256], zt[:, h * 256 : (h + 1) * 256]
                )

            # ---- reorder copy: free (v, y0, j) -> st2 (y0, h, j, v) ----
            st2 = s2pool.tile([128, 512], mybir.dt.float32)
            for h in range(2):
                eng = nc.gpsimd
                for y0 in range(2):
                    cin = st[:, h * 256 : (h + 1) * 256].rearrange(
                        "p (v yz j) -> p yz v j", v=8, yz=2, j=16
                    )[:, y0 : y0 + 1, :, :]
                    base2 = y0 * 256 + h * 128
                    cout = st2[:, base2 : base2 + 128].rearrange(
                        "p (o j v) -> p o v j", o=1, j=16, v=8
                    )
                    if eng is nc.scalar:
                        eng.mul(cout, cin, 1.0)
                    else:
                        eng.tensor_copy(cout, cin)

            # ---- store per Y-group: rows = 8q + 2Y + y0, full 1KB runs ----
            engs = [nc.sync, nc.scalar, nc.gpsimd, nc.gpsimd]
            for Y in range(4):
                base = img * H + 2 * Y
                if img < IMGS - 1:
                    # 256-row slice overhangs into the next image's rows but
                    # only rows 8q + 2Y + {0,1} are written.
                    ssrc = st2[32 * Y : 32 * Y + 32, :].rearrange(
                        "q (y0 c) -> q y0 c", y0=2, c=256
                    )
                    sdst = o[base : base + 256, :].rearrange(
                        "(q w) c -> q w c", w=8
                    )[:, 0:2, :]
                    engs[Y].dma_start(sdst, ssrc)
                else:
                    for y0 in range(2):
                        ssrc = st2[32 * Y : 32 * Y + 32, 256 * y0 : 256 * y0 + 256]
                        sdst = o[base + y0 : base + y0 + 249 : 8, :]
                        engs[Y].dma_start(sdst, ssrc)


def _build_nc():
    nc = bacc.Bacc(
        "TRN2", target_bir_lowering=False, debug=False, num_devices=NCORES
    )
    x_ap = nc.dram_tensor("x", [ROWS, W], mybir.dt.float32, kind="ExternalInput").ap()
    bd_ap = nc.dram_tensor(
        "bd", [128, 128], mybir.dt.float32, kind="ExternalInput"
    ).ap()
    o_ap = nc.dram_tensor("o", [ROWS, W], mybir.dt.float32, kind="ExternalOutput").ap()
    with tile.TileContext(nc) as tc:
        _dct_kernel(tc, o_ap, x_ap, bd_ap)
    nc.compile()
    return nc


def _make_bd(dct_basis: np.ndarray) -> np.ndarray:
    a = dct_basis.astype(np.float64) @ dct_basis.astype(np.float64)
    a = a.astype(np.float32)
    bd = np.zeros((128, 128), dtype=np.float32)
    for g in range(16):
        for x in range(P):
            for v in range(P):
                bd[8 * g + x, 16 * v + g] = a[v, x]
    return bd


def kernel(x: np.ndarray, dct_basis: np.ndarray) -> np.ndarray:
    global _nc_cache, LAST_RESULTS
    x = np.asarray(x, dtype=np.float32)
    dct_basis = np.asarray(dct_basis, dtype=np.float32)
    assert x.shape == (B, C, H, W)

    if _nc_cache is None:
        _nc_cache = _build_nc()
    nc = _nc_cache

    bd = _make_bd(dct_basis)
    in_maps = []
    for i in range(NCORES):
        xs = np.ascontiguousarray(x[i * BPC : (i + 1) * BPC]).reshape(ROWS, W)
        in_maps.append({"x": xs, "bd": bd})

    if TRACE:
        _ensure_ntff_hook()
    try:
        res = run_bass_kernel_spmd(
            nc, in_maps, core_ids=list(range(NCORES)), trace=TRACE
        )
    except ModuleNotFoundError:
        res = run_bass_kernel_spmd(
            nc, in_maps, core_ids=list(range(NCORES)), trace=False
        )
    LAST_RESULTS = res

    out = np.empty((B, C, H, W), dtype=np.float32)
    for i in range(NCORES):
        out[i * BPC : (i + 1) * BPC] = res.results[i]["o"].reshape(BPC, C, H, W)
    return out


# revision 12
# speedup vs baseline: 4.0665x; 1.9023x over previous
"""Trainium2 Bass kernel for nn_DCTLayer: per-8x8-block 2D DCT-like transform.

Math: reference computes, per 8x8 block X of the 256x256 image,
    out_block[y, v] = sum_x A[v, x] * X[x, y],   where A = D @ D
(D = 8x8 DCT basis). out_block = (A @ X)^T.

Kernel strategy (per core, pure data parallel over batch):
  - Load 128 consecutive image rows into SBUF naturally: partition = (G, x)
    [p = 8G+x], free = (r, c) [r = row-half, c = column]. Contiguous 1KB-run
    DMA.
  - Matmul per (r, h): stationary = data with free AP ordered (y, j)
    [c = 128h + 8j + y], moving = constant BD2[8g+x, 16v+g] = A[v,x].
    PSUM out: partition (y, j) [p = 16y+j], written STRIDED so the h-half of
    PSUM has free layout f = 32v + 16r + g.
  - DVE stream-transpose (32x32 blocks) per h-half: swaps partition-lo5
    (y0, j) with free-lo5 (r, g), yielding partition p' = 32Y + 16r + g
    [Y = y>>1], free (v, y0, j).  This is the cross-partition shuffle that
    makes output rows contiguous per partition.
  - Reorder copy (v, y0, j) -> st2 layout (y0, h, j, v): now partition p'
    holds output rows 8q + 2Y + y0 (q = p' mod 32) as full 1KB spans.
  - Store per Y-group: 3-dim AP [q(32 partitions), y0, 1KB row run];
    row = 8q + 2Y + y0 is affine in the partition index.  4 store DMAs per
    image with 1KB contiguous runs (full DMA bandwidth) vs the old 16 DMAs
    of 32B runs.
"""

import sys

sys.path.insert(0, "/opt/trn_rl_repo")

from contextlib import ExitStack

import numpy as np

import concourse.bass as bass  # noqa: F401
import concourse.tile as tile
from concourse import bacc, mybir
from concourse.bass_utils import run_bass_kernel_spmd

P = 8
H = W = 256
B, C = 16, 64
NCORES = 8
BPC = B // NCORES  # batches per core
IMGS = BPC * C  # images (b,c planes) per core
ROWS = IMGS * H  # dram rows per core

TRACE = False
LAST_RESULTS = None

_nc_cache = None


def _ensure_ntff_hook():
    """The agent image's antenv lacks axon_hooks; synthesize it so
    run_bass_kernel_spmd(trace=True) can capture NTFF profiles."""
    import types

    if "antenv.axon_hooks" in sys.modules:
        return
    try:
        sys.path.insert(0, "/root/.axon_site/trn_agent_boot")
        from trn_boot import _ntff_profile_via_ctypes

        hook = _ntff_profile_via_ctypes("/opt/axon/libaxon_pjrt.so")
    except Exception:
        hook = None
    mod = types.ModuleType("antenv.axon_hooks")
    mod._hook = hook
    mod.get_axon_ntff_profile_hook = lambda: mod._hook
    mod.set_axon_ntff_profile_hook = lambda h: setattr(mod, "_hook", h)
    sys.modules["antenv.axon_hooks"] = mod


def _dct_kernel(tc, o, x, bd):
    nc = tc.nc
    with ExitStack() as ctx:
        xpool = ctx.enter_context(tc.tile_pool(name="xin", bufs=6))
        x2pool = ctx.enter_context(tc.tile_pool(name="xperm", bufs=4))
        zpool = ctx.enter_context(tc.tile_pool(name="zint", bufs=4))
        spool = ctx.enter_context(tc.tile_pool(name="stt", bufs=4))
        s2pool = ctx.enter_context(tc.tile_pool(name="st2", bufs=4))
        cpool = ctx.enter_context(tc.tile_pool(name="const", bufs=1))
        ppool = ctx.enter_context(tc.tile_pool(name="ps", bufs=4, space="PSUM"))

        bdt = cpool.tile([128, 128], mybir.dt.float32)
        nc.sync.dma_start(bdt[:], bd[:])

        for img in range(IMGS):
            # ---- load image (256x256) as [p=(G,x), (r, c)]; 1KB runs ----
            xt = xpool.tile([128, 2 * W], mybir.dt.float32)
            src = x[img * H : (img + 1) * H, :].rearrange("(r p) c -> p r c", p=128)
            dst = xt[:].rearrange("p (r c) -> p r c", c=W)
            nc.sync.dma_start(dst, src)

            # ---- pre-permute cols to y-major so matmul APs stay 1-D ----
            # xt  free: (r, h, j, y)   ->   xt2 free: (r, h, y, j)
            xt2 = x2pool.tile([128, 2 * W], mybir.dt.float32)
            for r in range(2):
                cin = xt[:, r * W : (r + 1) * W].rearrange(
                    "p (h j y) -> p h y j", h=2, j=16, y=8
                )
                cout = xt2[:, r * W : (r + 1) * W].rearrange(
                    "p (h y j) -> p h y j", h=2, y=8, j=16
                )
                nc.vector.tensor_copy(cout, cin)

            # ---- 4 plain matmuls: psum chunk at 256h+128r = [p=(y,j), (v,g)] ----
            ps = ppool.tile([128, 512], mybir.dt.float32)
            for r in range(2):
                for h in range(2):
                    stat = xt2[:, r * W + h * 128 : r * W + (h + 1) * 128]
                    pout = ps[:, h * 256 + r * 128 : h * 256 + r * 128 + 128]
                    nc.tensor.matmul(pout, stat, bdt[:], start=True, stop=True)

            # ---- interleave copy per h: psum (r, v, g) -> zt (v, r, g) ----
            zt = zpool.tile([128, 512], mybir.dt.float32)
            for h in range(2):
                cin = ps[:, h * 256 : (h + 1) * 256].rearrange(
                    "p (r v g) -> p r v g", r=2, v=8, g=16
                )
                cout = zt[:, h * 256 : (h + 1) * 256].rearrange(
                    "p (v r g) -> p r v g", v=8, r=2, g=16
                )
                nc.scalar.mul(cout, cin, 1.0)

            # ---- DVE stream transpose, both halves in one instr ----
            # in:  partition (Y, y0, j), free (h, v, r, g)
            # out: partition (Y, r, g),  free (h, v, y0, j)
            st = spool.tile([128, 512], mybir.dt.float32)
            nc.vector.transpose(st[:], zt[:])

            # ---- reorder copy: free (v, y0, j) -> st2 (y0, h, j, v) ----
            st2 = s2pool.tile([128, 512], mybir.dt.float32)
            for h in range(2):
                eng = nc.vector if h == 1 else nc.scalar
                for y0 in range(2):
                    cin = st[:, h * 256 : (h + 1) * 256].rearrange(
                        "p (v yz j) -> p yz v j", v=8, yz=2, j=16
                    )[:, y0 : y0 + 1, :, :]
                    base2 = y0 * 256 + h * 128
                    cout = st2[:, base2 : base2 + 128].rearrange(
                        "p (o j v) -> p o v j", o=1, j=16, v=8
                    )
                    if eng is nc.scalar:
                        eng.mul(cout, cin, 1.0)
                    else:
                        eng.tensor_copy(cout, cin)

            # ---- one store per image: dram row' = 2*p' + y0 (host unshuffles)
            ssrc = st2[:].rearrange("p (y0 c) -> p y0 c", y0=2, c=256)
            sdst = o[img * H : (img + 1) * H, :].rearrange(
                "(q y0) c -> q y0 c", y0=2
            )
            nc.gpsimd.dma_start(sdst, ssrc)


def _build_nc():
    nc = bacc.Bacc(
        "TRN2", target_bir_lowering=False, debug=False, num_devices=NCORES
    )
    x_ap = nc.dram_tensor("x", [ROWS, W], mybir.dt.float32, kind="ExternalInput").ap()
    bd_ap = nc.dram_tensor(
        "bd", [128, 128], mybir.dt.float32, kind="ExternalInput"
    ).ap()
    o_ap = nc.dram_tensor("o", [ROWS, W], mybir.dt.float32, kind="ExternalOutput").ap()
    with tile.TileContext(nc) as tc:
        _dct_kernel(tc, o_ap, x_ap, bd_ap)
    nc.compile()
    return nc


def _make_bd(dct_basis: np.ndarray) -> np.ndarray:
    a = dct_basis.astype(np.float64) @ dct_basis.astype(np.float64)
    a = a.astype(np.float32)
    bd = np.zeros((128, 128), dtype=np.float32)
    for g in range(16):
        for x in range(P):
            for v in range(P):
                bd[8 * g + x, 16 * v + g] = a[v, x]
    return bd


def kernel(x: np.ndarray, dct_basis: np.ndarray) -> np.ndarray:
    global _nc_cache, LAST_RESULTS
    x = np.asarray(x, dtype=np.float32)
    dct_basis = np.asarray(dct_basis, dtype=np.float32)
    assert x.shape == (B, C, H, W)

    if _nc_cache is None:
        _nc_cache = _build_nc()
    nc = _nc_cache

    bd = _make_bd(dct_basis)
    in_maps = []
    for i in range(NCORES):
        xs = np.ascontiguousarray(x[i * BPC : (i + 1) * BPC]).reshape(ROWS, W)
        in_maps.append({"x": xs, "bd": bd})

    if TRACE:
        _ensure_ntff_hook()
    try:
        res = run_bass_kernel_spmd(
            nc, in_maps, core_ids=list(range(NCORES)), trace=TRACE
        )
    except ModuleNotFoundError:
        res = run_bass_kernel_spmd(
            nc, in_maps, core_ids=list(range(NCORES)), trace=False
        )
    LAST_RESULTS = res

    out = np.empty((B, C, H, W), dtype=np.float32)
    for i in range(NCORES):
        # device rows are (img, Y, r, g, y0); true rows are (r, g, Y, y0)
        oc = res.results[i]["o"].reshape(IMGS, 4, 2, 16, 2, W)
        oc = oc.transpose(0, 2, 3, 1, 4, 5).reshape(BPC, C, H, W)
        out[i * BPC : (i + 1) * BPC] = oc
    return out


# revision 14
# speedup vs baseline: 4.2779x; 1.0520x over previous
"""Trainium2 Bass kernel for nn_DCTLayer: per-8x8-block 2D DCT-like transform.

Math: reference computes, per 8x8 block X of the 256x256 image,
    out_block[y, v] = sum_x A[v, x] * X[x, y],   where A = D @ D
(D = 8x8 DCT basis). out_block = (A @ X)^T.

Kernel strategy (per core, pure data parallel over batch):
  - Load 128 consecutive image rows into SBUF naturally: partition = (G, x)
    [p = 8G+x], free = (r, c) [r = row-half, c = column]. Contiguous 1KB-run
    DMA.
  - Matmul per (r, h): stationary = data with free AP ordered (y, j)
    [c = 128h + 8j + y], moving = constant BD2[8g+x, 16v+g] = A[v,x].
    PSUM out: partition (y, j) [p = 16y+j], written STRIDED so the h-half of
    PSUM has free layout f = 32v + 16r + g.
  - DVE stream-transpose (32x32 blocks) per h-half: swaps partition-lo5
    (y0, j) with free-lo5 (r, g), yielding partition p' = 32Y + 16r + g
    [Y = y>>1], free (v, y0, j).  This is the cross-partition shuffle that
    makes output rows contiguous per partition.
  - Reorder copy (v, y0, j) -> st2 layout (y0, h, j, v): now partition p'
    holds output rows 8q + 2Y + y0 (q = p' mod 32) as full 1KB spans.
  - Store per Y-group: 3-dim AP [q(32 partitions), y0, 1KB row run];
    row = 8q + 2Y + y0 is affine in the partition index.  4 store DMAs per
    image with 1KB contiguous runs (full DMA bandwidth) vs the old 16 DMAs
    of 32B runs.
"""

import sys

sys.path.insert(0, "/opt/trn_rl_repo")

from contextlib import ExitStack

import numpy as np

import concourse.bass as bass  # noqa: F401
import concourse.tile as tile
from concourse import bacc, mybir
from concourse.bass_utils import run_bass_kernel_spmd

P = 8
H = W = 256
B, C = 16, 64
NCORES = 8
BPC = B // NCORES  # batches per core
IMGS = BPC * C  # images (b,c planes) per core
ROWS = IMGS * H  # dram rows per core

TRACE = False
LAST_RESULTS = None

_nc_cache = None


def _ensure_ntff_hook():
    """The agent image's antenv lacks axon_hooks; synthesize it so
    run_bass_kernel_spmd(trace=True) can capture NTFF profiles."""
    import types

    if "antenv.axon_hooks" in sys.modules:
        return
    try:
        sys.path.insert(0, "/root/.axon_site/trn_agent_boot")
        from trn_boot import _ntff_profile_via_ctypes

        hook = _ntff_profile_via_ctypes("/opt/axon/libaxon_pjrt.so")
    except Exception:
        hook = None
    mod = types.ModuleType("antenv.axon_hooks")
    mod._hook = hook
    mod.get_axon_ntff_profile_hook = lambda: mod._hook
    mod.set_axon_ntff_profile_hook = lambda h: setattr(mod, "_hook", h)
    sys.modules["antenv.axon_hooks"] = mod


def _dct_kernel(tc, o, x, bd):
    nc = tc.nc
    with ExitStack() as ctx:
        xpool = ctx.enter_context(tc.tile_pool(name="xin", bufs=6))
        x2pool = ctx.enter_context(tc.tile_pool(name="xperm", bufs=4))
        zpool = ctx.enter_context(tc.tile_pool(name="zint", bufs=4))
        spool = ctx.enter_context(tc.tile_pool(name="stt", bufs=4))
        s2pool = ctx.enter_context(tc.tile_pool(name="st2", bufs=4))
        cpool = ctx.enter_context(tc.tile_pool(name="const", bufs=1))
        ppool = ctx.enter_context(tc.tile_pool(name="ps", bufs=4, space="PSUM"))

        bdt = cpool.tile([128, 128], mybir.dt.float32r)
        nc.gpsimd.dma_start(bdt[:], bd[:])

        for img in range(IMGS):
            # ---- load image (256x256) as [p=(G,x), (r, c)]; 1KB runs ----
            xt = xpool.tile([128, 2 * W], mybir.dt.float32)
            src = x[img * H : (img + 1) * H, :].rearrange("(r p) c -> p r c", p=128)
            dst = xt[:].rearrange("p (r c) -> p r c", c=W)
            nc.sync.dma_start(dst, src)

            # ---- pre-permute cols to y-major so matmul APs stay 1-D ----
            # xt  free: (r, h, j, y)   ->   xt2 free: (r, h, y, j)
            xt2 = x2pool.tile([128, 2 * W], mybir.dt.float32r)
            for r in range(2):
                cin = xt[:, r * W : (r + 1) * W].rearrange(
                    "p (h j y) -> p h y j", h=2, j=16, y=8
                )
                cout = xt2[:, r * W : (r + 1) * W].rearrange(
                    "p (h y j) -> p h y j", h=2, y=8, j=16
                )
                nc.vector.tensor_copy(cout, cin)

            # ---- 4 plain matmuls: psum chunk at 256h+128r = [p=(y,j), (v,g)] ----
            ps = ppool.tile([128, 512], mybir.dt.float32)
            for r in range(2):
                for h in range(2):
                    stat = xt2[:, r * W + h * 128 : r * W + (h + 1) * 128]
                    pout = ps[:, h * 256 + r * 128 : h * 256 + r * 128 + 128]
                    nc.tensor.matmul(pout, stat, bdt[:], start=True, stop=True)

            # ---- interleave copy per h: psum (r, v, g) -> zt (v, r, g) ----
            zt = zpool.tile([128, 512], mybir.dt.float32)
            for h in range(2):
                cin = ps[:, h * 256 : (h + 1) * 256].rearrange(
                    "p (r v g) -> p r v g", r=2, v=8, g=16
                )
                cout = zt[:, h * 256 : (h + 1) * 256].rearrange(
                    "p (v r g) -> p r v g", v=8, r=2, g=16
                )
                nc.scalar.mul(cout, cin, 1.0)

            # ---- DVE stream transpose, both halves in one instr ----
            # in:  partition (Y, y0, j), free (h, v, r, g)
            # out: partition (Y, r, g),  free (h, v, y0, j)
            st = spool.tile([128, 512], mybir.dt.float32)
            nc.vector.transpose(st[:], zt[:])

            # ---- reorder copy: free (v, y0, j) -> st2 (y0, h, j, v) ----
            st2 = s2pool.tile([128, 512], mybir.dt.float32)
            for h in range(2):
                eng = nc.vector if h == 1 else nc.scalar
                for y0 in range(2):
                    cin = st[:, h * 256 : (h + 1) * 256].rearrange(
                        "p (v yz j) -> p yz v j", v=8, yz=2, j=16
                    )[:, y0 : y0 + 1, :, :]
                    base2 = y0 * 256 + h * 128
                    cout = st2[:, base2 : base2 + 128].rearrange(
                        "p (o j v) -> p o v j", o=1, j=16, v=8
                    )
                    if eng is nc.scalar:
                        eng.mul(cout, cin, 1.0)
                    else:
                        eng.tensor_copy(cout, cin)

            # ---- one store per image: dram row' = 2*p' + y0 (host unshuffles)
            ssrc = st2[:].rearrange("p (y0 c) -> p y0 c", y0=2, c=256)
            sdst = o[img * H : (img + 1) * H, :].rearrange(
                "(q y0) c -> q y0 c", y0=2
            )
            nc.gpsimd.dma_start(sdst, ssrc)


def _build_nc():
    nc = bacc.Bacc(
        "TRN2", target_bir_lowering=False, debug=False, num_devices=NCORES
    )
    x_ap = nc.dram_tensor("x", [ROWS, W], mybir.dt.float32, kind="ExternalInput").ap()
    bd_ap = nc.dram_tensor(
        "bd", [128, 128], mybir.dt.float32, kind="ExternalInput"
    ).ap()
    o_ap = nc.dram_tensor("o", [ROWS, W], mybir.dt.float32, kind="ExternalOutput").ap()
    with tile.TileContext(nc) as tc:
        _dct_kernel(tc, o_ap, x_ap, bd_ap)
    nc.compile()
    return nc


def _make_bd(dct_basis: np.ndarray) -> np.ndarray:
    a = dct_basis.astype(np.float64) @ dct_basis.astype(np.float64)
    a = a.astype(np.float32)
    bd = np.zeros((128, 128), dtype=np.float32)
    for g in range(16):
        for x in range(P):
            for v in range(P):
                bd[8 * g + x, 16 * v + g] = a[v, x]
    return bd


def kernel(x: np.ndarray, dct_basis: np.ndarray) -> np.ndarray:
    global _nc_cache, LAST_RESULTS
    x = np.asarray(x, dtype=np.float32)
    dct_basis = np.asarray(dct_basis, dtype=np.float32)
    assert x.shape == (B, C, H, W)

    if _nc_cache is None:
        _nc_cache = _build_nc()
    nc = _nc_cache

    bd = _make_bd(dct_basis)
    in_maps = []
    for i in range(NCORES):
        xs = np.ascontiguousarray(x[i * BPC : (i + 1) * BPC]).reshape(ROWS, W)
        in_maps.append({"x": xs, "bd": bd})

    if TRACE:
        _ensure_ntff_hook()
    try:
        res = run_bass_kernel_spmd(
            nc, in_maps, core_ids=list(range(NCORES)), trace=TRACE
        )
    except ModuleNotFoundError:
        res = run_bass_kernel_spmd(
            nc, in_maps, core_ids=list(range(NCORES)), trace=False
        )
    LAST_RESULTS = res

    out = np.empty((B, C, H, W), dtype=np.float32)
    for i in range(NCORES):
        # device rows are (img, Y, r, g, y0); true rows are (r, g, Y, y0)
        oc = res.results[i]["o"].reshape(IMGS, 4, 2, 16, 2, W)
        oc = oc.transpose(0, 2, 3, 1, 4, 5).reshape(BPC, C, H, W)
        out[i * BPC : (i + 1) * BPC] = oc
    return out


# revision 15
# speedup vs baseline: 4.3393x; 1.0144x over previous
"""Trainium2 Bass kernel for nn_DCTLayer: per-8x8-block 2D DCT-like transform.

Math: reference computes, per 8x8 block X of the 256x256 image,
    out_block[y, v] = sum_x A[v, x] * X[x, y],   where A = D @ D
(D = 8x8 DCT basis). out_block = (A @ X)^T.

Kernel strategy (per core, pure data parallel over batch):
  - Load 128 consecutive image rows into SBUF naturally: partition = (G, x)
    [p = 8G+x], free = (r, c) [r = row-half, c = column]. Contiguous 1KB-run
    DMA.
  - Matmul per (r, h): stationary = data with free AP ordered (y, j)
    [c = 128h + 8j + y], moving = constant BD2[8g+x, 16v+g] = A[v,x].
    PSUM out: partition (y, j) [p = 16y+j], written STRIDED so the h-half of
    PSUM has free layout f = 32v + 16r + g.
  - DVE stream-transpose (32x32 blocks) per h-half: swaps partition-lo5
    (y0, j) with free-lo5 (r, g), yielding partition p' = 32Y + 16r + g
    [Y = y>>1], free (v, y0, j).  This is the cross-partition shuffle that
    makes output rows contiguous per partition.
  - Reorder copy (v, y0, j) -> st2 layout (y0, h, j, v): now partition p'
    holds output rows 8q + 2Y + y0 (q = p' mod 32) as full 1KB spans.
  - Store per Y-group: 3-dim AP [q(32 partitions), y0, 1KB row run];
    row = 8q + 2Y + y0 is affine in the partition index.  4 store DMAs per
    image with 1KB contiguous runs (full DMA bandwidth) vs the old 16 DMAs
    of 32B runs.
"""

import sys

sys.path.insert(0, "/opt/trn_rl_repo")

from contextlib import ExitStack

import numpy as np

import concourse.bass as bass  # noqa: F401
import concourse.tile as tile
from concourse import bacc, mybir
from concourse.bass_utils import run_bass_kernel_spmd

P = 8
H = W = 256
B, C = 16, 64
NCORES = 8
BPC = B // NCORES  # batches per core
IMGS = BPC * C  # images (b,c planes) per core
ROWS = IMGS * H  # dram rows per core

TRACE = False
LAST_RESULTS = None

_nc_cache = None


def _ensure_ntff_hook():
    """The agent image's antenv lacks axon_hooks; synthesize it so
    run_bass_kernel_spmd(trace=True) can capture NTFF profiles."""
    import types

    if "antenv.axon_hooks" in sys.modules:
        return
    try:
        sys.path.insert(0, "/root/.axon_site/trn_agent_boot")
        from trn_boot import _ntff_profile_via_ctypes

        hook = _ntff_profile_via_ctypes("/opt/axon/libaxon_pjrt.so")
    except Exception:
        hook = None
    mod = types.ModuleType("antenv.axon_hooks")
    mod._hook = hook
    mod.get_axon_ntff_profile_hook = lambda: mod._hook
    mod.set_axon_ntff_profile_hook = lambda h: setattr(mod, "_hook", h)
    sys.modules["antenv.axon_hooks"] = mod


def _dct_kernel(tc, o, x, bd):
    nc = tc.nc
    with ExitStack() as ctx:
        xpool = ctx.enter_context(tc.tile_pool(name="xin", bufs=6))
        x2pool = ctx.enter_context(tc.tile_pool(name="xperm", bufs=4))
        zpool = ctx.enter_context(tc.tile_pool(name="zint", bufs=4))
        spool = ctx.enter_context(tc.tile_pool(name="stt", bufs=4))
        s2pool = ctx.enter_context(tc.tile_pool(name="st2", bufs=4))
        cpool = ctx.enter_context(tc.tile_pool(name="const", bufs=1))
        ppool = ctx.enter_context(tc.tile_pool(name="ps", bufs=4, space="PSUM"))

        bdt = cpool.tile([128, 128], mybir.dt.float32)
        nc.sync.dma_start(bdt[:], bd[:])

        for img in range(IMGS):
            # ---- load image (256x256) as [p=(G,x), (r, c)]; 1KB runs ----
            xt = xpool.tile([128, 2 * W], mybir.dt.float32)
            src = x[img * H : (img + 1) * H, :].rearrange("(r p) c -> p r c", p=128)
            dst = xt[:].rearrange("p (r c) -> p r c", c=W)
            nc.sync.dma_start(dst, src)

            # ---- pre-permute cols to y-major so matmul APs stay 1-D ----
            # xt  free: (r, h, j, y)   ->   xt2 free: (r, h, y, j)
            xt2 = x2pool.tile([128, 2 * W], mybir.dt.float32)
            for r in range(2):
                cin = xt[:, r * W : (r + 1) * W].rearrange(
                    "p (h j y) -> p h y j", h=2, j=16, y=8
                )
                cout = xt2[:, r * W : (r + 1) * W].rearrange(
                    "p (h y j) -> p h y j", h=2, y=8, j=16
                )
                nc.vector.tensor_copy(cout, cin)

            # ---- 4 plain matmuls: psum chunk at 256h+128r = [p=(y,j), (v,g)] ----
            ps = ppool.tile([128, 512], mybir.dt.float32)
            for r in range(2):
                for h in range(2):
                    stat = xt2[:, r * W + h * 128 : r * W + (h + 1) * 128]
                    pout = ps[:, h * 256 + r * 128 : h * 256 + r * 128 + 128]
                    nc.tensor.matmul(pout, stat, bdt[:], start=True, stop=True)

            # ---- interleave copy per h: psum (r, v, g) -> zt (v, r, g) ----
            zt = zpool.tile([128, 512], mybir.dt.float32)
            for h in range(2):
                cin = ps[:, h * 256 : (h + 1) * 256].rearrange(
                    "p (r v g) -> p r v g", r=2, v=8, g=16
                )
                cout = zt[:, h * 256 : (h + 1) * 256].rearrange(
                    "p (v r g) -> p r v g", v=8, r=2, g=16
                )
                nc.scalar.mul(cout, cin, 1.0)

            # ---- DVE stream transpose, both halves in one instr ----
            # in:  partition (Y, y0, j), free (h, v, r, g)
            # out: partition (Y, r, g),  free (h, v, y0, j)
            st = spool.tile([128, 512], mybir.dt.float32)
            nc.vector.transpose(st[:], zt[:])

            # ---- reorder copy: free (v, y0, j) -> st2 (y0, h, j, v) ----
            st2 = s2pool.tile([128, 512], mybir.dt.float32)
            for h in range(2):
                eng = nc.vector if h == 1 else nc.scalar
                for y0 in range(2):
                    cin = st[:, h * 256 : (h + 1) * 256].rearrange(
                        "p (v yz j) -> p yz v j", v=8, yz=2, j=16
                    )[:, y0 : y0 + 1, :, :]
                    base2 = y0 * 256 + h * 128
                    cout = st2[:, base2 : base2 + 128].rearrange(
                        "p (o j v) -> p o v j", o=1, j=16, v=8
                    )
                    if eng is nc.scalar:
                        eng.mul(cout, cin, 1.0)
                    else:
                        eng.tensor_copy(cout, cin)

            # ---- one store per image: dram row' = 2*p' + y0 (host unshuffles)
            ssrc = st2[:].rearrange("p (y0 c) -> p y0 c", y0=2, c=256)
            sdst = o[img * H : (img + 1) * H, :].rearrange(
                "(q y0) c -> q y0 c", y0=2
            )
            nc.gpsimd.dma_start(sdst, ssrc)


def _build_nc():
    nc = bacc.Bacc(
        "TRN2", target_bir_lowering=False, debug=False, num_devices=NCORES
    )
    x_ap = nc.dram_tensor("x", [ROWS, W], mybir.dt.float32, kind="ExternalInput").ap()
    bd_ap = nc.dram_tensor(
        "bd", [128, 128], mybir.dt.float32, kind="ExternalInput"
    ).ap()
    o_ap = nc.dram_tensor("o", [ROWS, W], mybir.dt.float32, kind="ExternalOutput").ap()
    with tile.TileContext(nc) as tc:
        _dct_kernel(tc, o_ap, x_ap, bd_ap)
    nc.compile()
    return nc


def _make_bd(dct_basis: np.ndarray) -> np.ndarray:
    a = dct_basis.astype(np.float64) @ dct_basis.astype(np.float64)
    a = a.astype(np.float32)
    bd = np.zeros((128, 128), dtype=np.float32)
    for g in range(16):
        for x in range(P):
            for v in range(P):
                bd[8 * g + x, 16 * v + g] = a[v, x]
    return bd


def kernel(x: np.ndarray, dct_basis: np.ndarray) -> np.ndarray:
    global _nc_cache, LAST_RESULTS
    x = np.asarray(x, dtype=np.float32)
    dct_basis = np.asarray(dct_basis, dtype=np.float32)
    assert x.shape == (B, C, H, W)

    if _nc_cache is None:
        _nc_cache = _build_nc()
    nc = _nc_cache

    bd = _make_bd(dct_basis)
    in_maps = []
    for i in range(NCORES):
        xs = np.ascontiguousarray(x[i * BPC : (i + 1) * BPC]).reshape(ROWS, W)
        in_maps.append({"x": xs, "bd": bd})

    if TRACE:
        _ensure_ntff_hook()
    try:
        res = run_bass_kernel_spmd(
            nc, in_maps, core_ids=list(range(NCORES)), trace=TRACE
        )
    except ModuleNotFoundError:
        res = run_bass_kernel_spmd(
            nc, in_maps, core_ids=list(range(NCORES)), trace=False
        )
    LAST_RESULTS = res

    out = np.empty((B, C, H, W), dtype=np.float32)
    for i in range(NCORES):
        # device rows are (img, Y, r, g, y0); true rows are (r, g, Y, y0)
        oc = res.results[i]["o"].reshape(IMGS, 4, 2, 16, 2, W)
        oc = oc.transpose(0, 2, 3, 1, 4, 5).reshape(BPC, C, H, W)
        out[i * BPC : (i + 1) * BPC] = oc
    return out
